# revision 1
# baseline (speedup 1.0000x reference)
"""Bass/Trainium2 kernel for DirSAGEEmbRes (2-layer directed SAGE + residual).

Strategy (8 NeuronCores, SPMD):
  - dst nodes sharded 1/8 per core; edge lists bucketed per (core, dst-window,
    src-quarter) on host, padded so all cores share one compile-time layout.
  - segment-mean aggregation: dma_gather (bf16 message rows) + PE one-hot
    scatter matmuls into PSUM windows; counts folded in as per-partition
    1/deg multipliers on eviction.
  - dense SAGE transforms run feature-major (weights stationary as lhsT).
  - h1 exchanged between cores via AllGather; layer-2 gathers index the
    gathered table.
  - y_in / y_out computed per direction and summed on device.
"""
import os
import sys

sys.path.insert(0, "/opt/trn_rl_repo")

import numpy as np
import ml_dtypes

import concourse.bass as bass
import concourse.bacc as bacc
import concourse.mybir as mybir
from concourse.tile import TileContext
from concourse.library_config import mlp
from concourse.masks import make_identity

BF = ml_dtypes.bfloat16
NCORE = 8
GW = 9          # dst windows per reduce group (3 PSUM bank-tiles of 3)
HID = 128
D0 = 144


# ----------------------------------------------------------------- host prep

def _ceil(a, b):
    return -(-a // b)


class PassLayout:
    """Compile-time layout of one aggregation pass (shared by all cores)."""

    def __init__(self, L_wq, W):
        # L_wq: [W, 4] per-(window, quarter) run lengths (max over cores)
        self.W = W
        self.NG = _ceil(W, GW)
        self.L_wq = L_wq
        self.segs = []          # (g, q, idx_col16, seglen, dst_col0, mms)
        self.run_start = np.zeros((W, 4), np.int64)  # pos of run within its segment
        self.seg_start = {}      # (g, q) -> (idx stream position, dst col0)
        idx_pos = 0
        dst_col = 0
        for g in range(self.NG):
            wins = list(range(g * GW, min((g + 1) * GW, W)))
            for q in range(4):
                runs = [int(L_wq[w, q]) for w in wins]
                total = int(np.sum(runs))
                seglen = _ceil(max(total, 1), 128) * 128
                # run offsets within segment
                off = 0
                spans = []
                for w, r in zip(wins, runs):
                    self.run_start[w, q] = off
                    spans.append((w, off, off + r))
                    off += r
                # matmul list: (block, window, col)
                mms = []
                col = 0
                for b in range(seglen // 128):
                    lo, hi = b * 128, b * 128 + 128
                    for (w, s, e) in spans:
                        if s < hi and e > lo:
                            mms.append((b, w, col))
                            col += 1
                self.seg_start[(g, q)] = (idx_pos, dst_col)
                self.segs.append((g, q, idx_pos, seglen, dst_col, mms))
                idx_pos += seglen
                dst_col += col
        self.tot_idx = idx_pos
        self.tot_col = dst_col


def _build_pass(src_q, src_iq, core, w, woff, layout, npcore_edges):
    """Build per-core idx (int16, wrap layout) + dstoff (bf16) streams."""
    W = layout.W
    idx_streams = np.zeros((NCORE, layout.tot_idx), np.int16)
    dst_streams = np.full((NCORE, layout.tot_col, 128), -1.0, np.float32)
    order = np.lexsort((src_iq, src_q, w, core))
    c_s, w_s, q_s, iq_s, woff_s = (
        core[order], w[order], src_q[order], src_iq[order], woff[order])
    key = ((c_s * W + w_s) * 4 + q_s).astype(np.int64)
    # rank within (core, w, q)
    uniq, first = np.unique(key, return_index=True)
    rank = np.arange(len(key)) - first[np.searchsorted(uniq, key)]
    g_s = w_s // GW
    seg0 = np.zeros(len(key), np.int64)
    for (g, q), (ip, dc) in layout.seg_start.items():
        m = (g_s == g) & (q_s == q)
        seg0[m] = ip
    pos_in_seg = layout.run_start[w_s, q_s] + rank
    pos = seg0 + pos_in_seg
    for c in range(NCORE):
        m = c_s == c
        idx_streams[c, pos[m]] = iq_s[m].astype(np.int16)
    # dst offsets: map (g, q, block, w) -> col
    colmap = {}
    for (g, q, ip, seglen, dc, mms) in layout.segs:
        for (b, wv, col) in mms:
            colmap[(g, q, b, wv)] = dc + col
    bl = pos_in_seg // 128
    eoff = pos_in_seg % 128
    NG = layout.NG
    maxb = max((s[3] // 128 for s in layout.segs), default=1)
    colarr = np.full((NG, 4, maxb, W), -1, np.int64)
    for (g, q, ip, seglen, dc, mms) in layout.segs:
        for (b, wv, col) in mms:
            colarr[g, q, b, wv] = dc + col
    cols = colarr[g_s, q_s, bl, w_s]
    assert (cols >= 0).all()
    for c in range(NCORE):
        m = c_s == c
        dst_streams[c, cols[m], eoff[m]] = woff_s[m].astype(np.float32)
    # wrap idx: unwrapped[i] = wrap[i%16, i//16], tiled to 128 partitions
    idx_wrap = np.ascontiguousarray(
        np.tile(idx_streams.reshape(NCORE, layout.tot_idx // 16, 16)
                .transpose(0, 2, 1), (1, 8, 1)))
    dst_t = np.ascontiguousarray(dst_streams.transpose(0, 2, 1))  # [8,128,cols]
    return idx_wrap, dst_t


def _preprocess(edge, N, NPC, NPCP, W, QS, QSH):
    """Per direction: build L1/L2 layouts + streams + inv-count tables."""
    src = edge[0].astype(np.int64)
    dst = edge[1].astype(np.int64)
    core = dst // NPC
    ldst = dst - core * NPC
    w = ldst // 128
    woff = ldst % 128

    # inverse counts (0 for deg-0), [8, 128, W]
    cnt = np.bincount(core * NPCP + ldst, minlength=NCORE * NPCP).reshape(NCORE, NPCP)
    inv = np.where(cnt > 0, 1.0 / np.maximum(cnt, 1), 0.0).astype(np.float32)
    inv_t = np.ascontiguousarray(
        inv.reshape(NCORE, W, 128).transpose(0, 2, 1))  # [8, 128, W]

    out = {}
    for layer in (1, 2):
        if layer == 1:
            q = src // QS
            iq = src - q * QS
        else:
            pos = (src // NPC) * NPCP + (src % NPC)
            q = pos // QSH
            iq = pos - q * QSH
        counts = np.bincount(((core * W + w) * 4 + q).astype(np.int64),
                             minlength=NCORE * W * 4).reshape(NCORE, W, 4)
        layout = PassLayout(counts.max(axis=0), W)
        idx_wrap, dst_t = _build_pass(q, iq, core, w, woff, layout, None)
        out[layer] = (layout, idx_wrap, dst_t)
    return out, inv_t


# -------------------------------------------------------------- bass builder

def _build_nc(N, NPC, NPCP, W, QS, QSH, NT, NTH, layouts):
    """layouts: dict[(dir, layer)] -> PassLayout"""
    f32 = mybir.dt.float32
    bf16 = mybir.dt.bfloat16
    XE, HE = 256, 128  # table row elems (bf16)
    M2 = 2 * NPCP  # x0T cols: [0:NPCP]=dims 0-127, [NPCP:]=dims 128-143 on rows 0:16

    nc = bacc.Bacc(None, target_bir_lowering=False, debug=False,
                   num_swdge_queues=2)
    t_xt = nc.dram_tensor("xt", [NT, XE], bf16, kind="ExternalInput")
    t_x0T = nc.dram_tensor("x0T", [128, M2], f32, kind="ExternalInput")
    t_iota = nc.dram_tensor("iota", [128, 128], bf16, kind="ExternalInput")
    t_idx, t_dst, t_inv = {}, {}, {}
    for d in range(2):
        for layer in (1, 2):
            lo = layouts[(d, layer)]
            t_idx[(d, layer)] = nc.dram_tensor(
                f"idx{layer}_{d}", [128, lo.tot_idx // 16], mybir.dt.int16,
                kind="ExternalInput")
            t_dst[(d, layer)] = nc.dram_tensor(
                f"dst{layer}_{d}", [128, lo.tot_col], f32, kind="ExternalInput")
        t_inv[d] = nc.dram_tensor(f"inv_{d}", [128, W], f32, kind="ExternalInput")
    names = ["in1", "in2", "out1", "out2"]
    t_w = {}
    for i, nm in enumerate(names):
        d_in = D0 if nm.endswith("1") else HID
        t_w[nm + "_Wl"] = nc.dram_tensor(nm + "_Wl", [d_in, HID], f32, kind="ExternalInput")
        t_w[nm + "_Wr"] = nc.dram_tensor(nm + "_Wr", [d_in, HID], f32, kind="ExternalInput")
        t_w[nm + "_bl"] = nc.dram_tensor(nm + "_bl", [HID], f32, kind="ExternalInput")
    t_lin = nc.dram_tensor("lin_W", [2 * HID], f32, kind="ExternalInput")
    t_linb = nc.dram_tensor("lin_b", [128], f32, kind="ExternalInput")
    t_y = nc.dram_tensor("y", [NPCP], f32, kind="ExternalOutput")

    NTILE = _ceil(NPCP, 512)

    with TileContext(nc) as tc:
        with (
            tc.tile_pool(name="const", bufs=1) as constp,
            tc.tile_pool(name="stage", bufs=3) as stagep,
            tc.tile_pool(name="small", bufs=3) as smallp,
            tc.tile_pool(name="sS", bufs=6) as sp,
            tc.tile_pool(name="big", bufs=1) as bigp,
            tc.tile_pool(name="ps", bufs=6, space="PSUM") as psp,
            tc.tile_pool(name="pst", bufs=2, space="PSUM") as pstp,
            tc.tile_pool(name="dram", bufs=1, space="DRAM") as dramp,
        ):
            nc.gpsimd.load_library(mlp)
            # constants
            ident = constp.tile([128, 128], f32, tag="ident")
            make_identity(nc, ident[:])
            ident_bf = constp.tile([128, 128], bf16, tag="identbf")
            nc.vector.tensor_copy(out=ident_bf[:], in_=ident[:])
            iota = constp.tile([128, 128], bf16, tag="iota")
            nc.sync.dma_start(out=iota[:], in_=t_iota[:])
            invt = {}
            for d in range(2):
                invt[d] = constp.tile([128, W], f32, tag=f"inv{d}", name=f"inv{d}")
                nc.sync.dma_start(out=invt[d][:], in_=t_inv[d][:])
            wt = {}
            for nm in names:
                for side in ("Wl", "Wr"):
                    src_t = t_w[nm + "_" + side]
                    a = constp.tile([128, 128], f32, tag=f"{nm}{side}a", name=f"{nm}{side}a")
                    nc.sync.dma_start(out=a[:], in_=src_t[0:128, :])
                    wt[nm + side + "a"] = a
                    if nm.endswith("1"):
                        b = constp.tile([128, 128], f32, tag=f"{nm}{side}b", name=f"{nm}{side}b")
                        nc.sync.dma_start(out=b[0:16, :], in_=src_t[128:144, :])
                        wt[nm + side + "b"] = b
                bt = constp.tile([128, 1], f32, tag=f"{nm}bl", name=f"{nm}bl")
                nc.sync.dma_start(out=bt[:], in_=t_w[nm + "_bl"][:, None])
                wt[nm + "bl"] = bt
            # bf16 copies for L2 Wr (rhs h1T is bf16)
            for nm in ("in2", "out2"):
                wb = constp.tile([128, 128], bf16, tag=f"{nm}Wrbf", name=f"{nm}Wrbf")
                nc.vector.tensor_copy(out=wb[:], in_=wt[nm + "Wra"][:])
                wt[nm + "Wrbf"] = wb
            for nm in ("in1", "out1"):
                wb = constp.tile([128, 128], bf16, tag=f"{nm}Wlbbf", name=f"{nm}Wlbbf")
                nc.vector.tensor_copy(out=wb[0:16, :], in_=wt[nm + "Wlb"][0:16, :])
                wt[nm + "Wlbbf"] = wb
            lin_f = constp.tile([128, 2], f32, tag="linf")
            nc.sync.dma_start(out=lin_f[:], in_=t_lin.rearrange("(h p) -> p h", p=128))
            lin_bf = constp.tile([128, 2], bf16, tag="linbf")
            nc.vector.tensor_copy(out=lin_bf[:], in_=lin_f[:])
            linb_sb = constp.tile([128, 1], f32, tag="linb")
            nc.sync.dma_start(out=linb_sb[:], in_=t_linb[:, None])

            y_sb = constp.tile([128, W], f32, tag="ysb")

            h1own = [dramp.tile([NPCP, HID], bf16, tag=f"h1own{d}", name=f"h1own{d}") for d in range(2)]
            h_ag = [dramp.tile([NCORE * NPCP, HID], bf16, tag=f"hag{d}",
                               name=f"hag{d}", addr_space="Shared") for d in range(2)]

            qctr = [0]

            def reduce_pass(d, layer, table_ap, QSZ, ELEM, DUse, meanT, meanT2=None):
                lo = layouts[(d, layer)]
                for g in range(lo.NG):
                    wins = list(range(g * GW, min((g + 1) * GW, lo.W)))
                    ptiles = [psp.tile([128, 512], f32, tag="red", name=f"red{g}_{k}") for k in range(3)]
                    for t in ptiles:
                        nc.vector.memset(t[:], 0.0)
                    segs = [s for s in lo.segs if s[0] == g]
                    # bank-level start/stop: first/last matmul per psum tile
                    mm_count = [0, 0, 0]
                    for (_, _, _, _, _, mms) in segs:
                        for (_, wv, _) in mms:
                            mm_count[wins.index(wv) // 3] += 1
                    mm_seen = [0, 0, 0]
                    for (gg, q, ip, seglen, dc, mms) in segs:
                        nb = seglen // 128
                        idxs = smallp.tile([128, seglen // 16], mybir.dt.int16,
                                           tag="idxs")
                        nc.sync.dma_start(
                            out=idxs[:],
                            in_=t_idx[(d, layer)][:, ip // 16: ip // 16 + seglen // 16])
                        ncols = len(mms)
                        dstc = smallp.tile([128, max(ncols, 1)], f32, tag="dstc")
                        if ncols:
                            nc.sync.dma_start(
                                out=dstc[:, 0:ncols],
                                in_=t_dst[(d, layer)][:, dc:dc + ncols])
                        stage = stagep.tile([128, nb * ELEM], bf16, tag="stage")
                        nc.gpsimd.dma_gather(
                            stage[:].rearrange("p (b e) -> p b e", e=ELEM),
                            table_ap[q * QSZ:(q + 1) * QSZ],
                            idxs[:],
                            seglen, seglen, ELEM,
                            single_packet=False,
                            queue_num=qctr[0] % 2,
                        )
                        qctr[0] += 1
                        for (b, wv, col) in mms:
                            wl = wins.index(wv)
                            k = wl // 3
                            S = sp.tile([128, 128], bf16, tag="S")
                            nc.vector.tensor_scalar(
                                out=S[:], in0=iota[:],
                                scalar1=dstc[:, col:col + 1], scalar2=None,
                                op0=mybir.AluOpType.is_equal)
                            mm_seen[k] += 1
                            nc.tensor.matmul(
                                ptiles[k][:, (wl % 3) * 160:(wl % 3) * 160 + DUse],
                                lhsT=S[:],
                                rhs=stage[:, b * ELEM:b * ELEM + DUse],
                                start=(mm_seen[k] == 1),
                                stop=(mm_seen[k] == mm_count[k]))
                    # evictions
                    for wl, wv in enumerate(wins):
                        mn = smallp.tile([128, DUse], f32, tag="meanN")
                        nc.vector.tensor_scalar(
                            out=mn[:],
                            in0=ptiles[wl // 3][:, (wl % 3) * 160:(wl % 3) * 160 + DUse],
                            scalar1=invt[d][:, wv:wv + 1], scalar2=None,
                            op0=mybir.AluOpType.mult)
                        tp = pstp.tile([128, 256], f32, tag="tr")
                        nc.tensor.transpose(tp[:, 0:128], mn[:, 0:128], ident[:])
                        nc.vector.tensor_copy(
                            out=meanT[:, wv * 128:(wv + 1) * 128], in_=tp[:, 0:128])
                        if DUse > 128:
                            nc.tensor.transpose(tp[0:16, 128:256], mn[:, 128:DUse],
                                                ident[:])
                            nc.vector.tensor_copy(
                                out=meanT2[0:16, wv * 128:(wv + 1) * 128],
                                in_=tp[0:16, 128:256])

            for d in range(2):
                nm1, nm2 = names[2 * d], names[2 * d + 1]
                # ---------------- layer 1 ----------------
                meanT = bigp.tile([128, NPCP], f32, tag="meanT")
                meanT2 = bigp.tile([128, NPCP], bf16, tag="mT2")
                reduce_pass(d, 1, t_xt[:], QS, 256, D0, meanT, meanT2)
                h1T = bigp.tile([128, NPCP], bf16, tag="h1T")
                for ti in range(NTILE):
                    n0 = ti * 512
                    nn = min(512, NPCP - n0)
                    xa = smallp.tile([128, 512], f32, tag="xa")
                    nc.sync.dma_start(out=xa[:, 0:nn], in_=t_x0T[:, n0:n0 + nn])
                    xb = smallp.tile([128, 512], f32, tag="xb")
                    nc.sync.dma_start(out=xb[0:16, 0:nn],
                                      in_=t_x0T[0:16, NPCP + n0:NPCP + n0 + nn])
                    ps = psp.tile([128, 512], f32, tag="red")
                    nc.tensor.matmul(ps[:, 0:nn], lhsT=wt[nm1 + "Wla"][:],
                                     rhs=meanT[:, n0:n0 + nn], start=True, stop=False)
                    nc.tensor.matmul(ps[:, 0:nn], lhsT=wt[nm1 + "Wlbbf"][0:16, :],
                                     rhs=meanT2[0:16, n0:n0 + nn],
                                     start=False, stop=False)
                    nc.tensor.matmul(ps[:, 0:nn], lhsT=wt[nm1 + "Wra"][:],
                                     rhs=xa[:, 0:nn], start=False, stop=False)
                    nc.tensor.matmul(ps[:, 0:nn], lhsT=wt[nm1 + "Wrb"][0:16, :],
                                     rhs=xb[0:16, 0:nn], start=False, stop=True)
                    nc.scalar.activation(out=h1T[:, n0:n0 + nn], in_=ps[:, 0:nn],
                                         func=mybir.ActivationFunctionType.Relu,
                                         bias=wt[nm1 + "bl"][:, 0:1], scale=1.0)
                # export h1 node-major bf16
                for w0 in range(0, W, 16):
                    wn = min(16, W - w0)
                    hn = smallp.tile([128, 16 * 128], bf16, tag="h1n")
                    for k in range(wn):
                        tp = pstp.tile([128, 256], bf16, tag="tr", name="tpbf")
                        nc.tensor.transpose(
                            tp[:, 0:128],
                            h1T[:, (w0 + k) * 128:(w0 + k + 1) * 128], ident_bf[:])
                        nc.vector.tensor_copy(out=hn[:, k * 128:(k + 1) * 128],
                                              in_=tp[:, 0:128])
                    nc.sync.dma_start(
                        out=h1own[d][:].rearrange("(w p) j -> p w j", p=128)[:, w0:w0 + wn, :],
                        in_=hn[:, 0:wn * 128].rearrange("p (w j) -> p w j", j=128))
                nc.gpsimd.collective_compute(
                    "AllGather", mybir.AluOpType.bypass,
                    replica_groups=[list(range(NCORE))],
                    ins=[h1own[d][:]], outs=[h_ag[d][:]])
                # ---------------- layer 2 ----------------
                mean2T = bigp.tile([128, NPCP], f32, tag="meanT")
                reduce_pass(d, 2, h_ag[d][:], QSH, 128, HID, mean2T)
                for ti in range(NTILE):
                    n0 = ti * 512
                    nn = min(512, NPCP - n0)
                    ps = psp.tile([128, 512], f32, tag="red")
                    nc.tensor.matmul(ps[:, 0:nn], lhsT=wt[nm2 + "Wla"][:],
                                     rhs=mean2T[:, n0:n0 + nn], start=True, stop=False)
                    nc.tensor.matmul(ps[:, 0:nn], lhsT=wt[nm2 + "Wrbf"][:],
                                     rhs=h1T[:, n0:n0 + nn], start=False, stop=True)
                    h2t = smallp.tile([128, 512], bf16, tag="h2t")
                    nc.scalar.activation(out=h2t[:, 0:nn], in_=ps[:, 0:nn],
                                         func=mybir.ActivationFunctionType.Relu,
                                         bias=wt[nm2 + "bl"][:, 0:1], scale=1.0)
                    # residual: hT = h1T + h2T (overwrite h1T)
                    nc.vector.tensor_add(out=h1T[:, n0:n0 + nn],
                                         in0=h1T[:, n0:n0 + nn], in1=h2t[:, 0:nn])
                # ---------------- y ----------------
                for wv in range(W):
                    yp = pstp.tile([128, 256], f32, tag="tr")
                    nc.tensor.matmul(yp[:, 0:1],
                                     lhsT=h1T[:, wv * 128:(wv + 1) * 128],
                                     rhs=lin_bf[:, d:d + 1], start=True, stop=True)
                    if d == 0:
                        nc.vector.tensor_copy(out=y_sb[:, wv:wv + 1], in_=yp[:, 0:1])
                    else:
                        nc.vector.tensor_add(out=y_sb[:, wv:wv + 1],
                                             in0=y_sb[:, wv:wv + 1], in1=yp[:, 0:1])
            # + lin_b, write out
            nc.vector.tensor_scalar(
                out=y_sb[:], in0=y_sb[:],
                scalar1=linb_sb[:, 0:1], scalar2=None,
                op0=mybir.AluOpType.add)
            nc.sync.dma_start(out=t_y.rearrange("(w p) -> p w", p=128), in_=y_sb[:])

    nc.compile()
    return nc


# ------------------------------------------------------------------ wrapper

def _prep_all(x, edge_in, edge_out, emb):
    N = x.shape[0]
    NPC = N // NCORE
    W = _ceil(NPC, 128)
    NPCP = W * 128
    QS = _ceil(_ceil(N, 4), 128) * 128
    NT = 4 * QS
    QSH = 2 * NPCP
    NTH = NCORE * NPCP

    x = np.asarray(x, np.float32)
    emb = np.asarray(emb, np.float32)
    xt = np.zeros((NT, 256), BF)
    xt[:N, 0:128] = x.astype(BF)
    xt[:N, 128:144] = emb.astype(BF)

    pre, invs = {}, {}
    for d, edge in enumerate((edge_in, edge_out)):
        pre[d], invs[d] = _preprocess(np.asarray(edge), N, NPC, NPCP, W, QS, QSH)

    M2 = 2 * NPCP
    x0T = np.zeros((NCORE, 128, M2), np.float32)
    for c in range(NCORE):
        blk = np.zeros((NPCP, D0), np.float32)
        blk[:NPC, 0:128] = x[c * NPC:(c + 1) * NPC]
        blk[:NPC, 128:144] = emb[c * NPC:(c + 1) * NPC]
        x0T[c, :, :NPCP] = blk[:, 0:128].T
        x0T[c, 0:16, NPCP:] = blk[:, 128:144].T

    iota = np.broadcast_to(np.arange(128, dtype=np.float32), (128, 128)).astype(BF)
    layouts = {(d, layer): pre[d][layer][0] for d in range(2) for layer in (1, 2)}
    dims = dict(N=N, NPC=NPC, NPCP=NPCP, W=W, QS=QS, QSH=QSH, NT=NT, NTH=NTH)
    return dims, layouts, pre, invs, xt, x0T, iota


def _in_maps(dims, pre, invs, xt, x0T, iota, kw):
    maps = []
    for c in range(NCORE):
        m = {"xt": xt, "x0T": np.ascontiguousarray(x0T[c]), "iota": np.asarray(iota)}
        for d in range(2):
            for layer in (1, 2):
                _, idx_wrap, dst_t = pre[d][layer]
                m[f"idx{layer}_{d}"] = np.ascontiguousarray(idx_wrap[c])
                m[f"dst{layer}_{d}"] = np.ascontiguousarray(dst_t[c])
            m[f"inv_{d}"] = np.ascontiguousarray(invs[d][c])
        for nm in ("in1", "in2", "out1", "out2"):
            m[nm + "_Wl"] = np.asarray(kw[nm + "_Wl"], np.float32)
            m[nm + "_Wr"] = np.asarray(kw[nm + "_Wr"], np.float32)
            m[nm + "_bl"] = np.asarray(kw[nm + "_bl"], np.float32)
        m["lin_W"] = np.asarray(kw["lin_W"], np.float32).reshape(-1)
        m["lin_b"] = np.full(128, np.asarray(kw["lin_b"], np.float32).reshape(-1)[0], np.float32)
        maps.append(m)
    return maps


def kernel(x, edge_in, edge_out, emb, **kw):
    from concourse.bass_utils import run_bass_kernel_spmd
    dims, layouts, pre, invs, xt, x0T, iota = _prep_all(x, edge_in, edge_out, emb)
    nc = _build_nc(dims["N"], dims["NPC"], dims["NPCP"], dims["W"], dims["QS"],
                   dims["QSH"], dims["NT"], dims["NTH"], layouts)
    maps = _in_maps(dims, pre, invs, xt, x0T, iota, kw)
    res = run_bass_kernel_spmd(nc, maps, core_ids=list(range(NCORE)))
    NPC = dims["NPC"]
    y = np.empty(dims["N"], np.float32)
    for c in range(NCORE):
        y[c * NPC:(c + 1) * NPC] = res.results[c]["y"][:NPC]
    return y

